# revision 45
# baseline (speedup 1.0000x reference)
"""Causal multi-head attention (RoPE) on 8 TRN2 NeuronCores.

Sharding: Megatron-style head parallelism. Each core owns 2 of the 16 heads:
it computes q/k/v projections for its 128 output features (2 heads x 64),
applies interleaved-pair RoPE (rotation done as a PE matmul with a constant
pair-swap matrix R, tables precomputed host-side), runs causal attention for
its (batch, head) pairs in the transposed orientation S^T = K^T Q so that no
on-chip transposes of the attention matrix are needed, and accumulates
attn^T-weighted V with an extra all-ones V column that yields the softmax
denominators for free. An AllToAll then redistributes the per-head outputs
from head-sharded to token-sharded layout, and each core computes the final
output projection for its 512-token slice. A tiny dummy AllToAll is issued at
kernel start to absorb the one-time collective warmup (~95us) while the
engines compute.

Perf structure (v3):
- Diagonal S^T/exp/AV chunks are column-trimmed to the causal window
  (queries >= first key of the chunk), cutting ACT-exp time and PE stream
  time; the residual intra-chunk wedge mask is a single shared [128,256]
  constant applied with a 3D AP.
- Softmax denominators: the [1,512]-row DVE reciprocal costs ~6 cyc/elem
  on a single partition (3.3us, 2x per qt, serial on DVE). Instead both
  hl den rows are ACT-copied to SBUF, PE-transposed into [128,8], one
  128-way-parallel reciprocal (~0.2us), transposed back column-by-column
  into a [1,1024] row (base-partition-0 constraint) feeding the bc
  broadcast matmuls.
- QKV bias-adds ride the ACT bias port (frees DVE for the rope muls).
- 4 sub-collectives of 1024 tokens. outproj2 is emitted before the tail
  collective so the scheduler interleaves it with qt3; tail og drains
  ride sync+scalar, keeping the gpsimd ring free for cin feeds +
  collective triggers. The remaining tail is bounded by ncfw a2a latency
  (~17-25us/collective, serialized on the CC cores; a split tail
  measured neutral because of that serialization).
- Per-qt AllToAll feed DMAs keep a dedicated gpsimd ring; the 2MB wo
  transfer is emitted after b0's first attention block so it doesn't sit
  ahead of the tt2 x tile on the scalar ring.
- Constant DMAs are spread across the scalar/gpsimd/sync queues so the
  first QKV matmul isn't gated on one serialized descriptor ring, and a
  burst of dummy matmuls at t=0 lifts the PE HAM clock gate to 2.4 GHz
  before real work lands. (Sustained rate is HAM power-throttled to
  ~50-81% duty for most of the run; total PE work is the binding floor.)
- PSUM: 2x[128,1024] sT pipeline (4 banks) + 4x[128,512] shared pool
  (attention accumulators, rot/bc/dT/rcP transients, QKV projection
  accumulator).

Compute dtype: bf16 operands with fp32 PSUM accumulation. (float32r would
be more precise, but f32r matmuls do not register as PE activity for the HAM
clock gate, so the PE stays throttled at 1.2 GHz; bf16 runs at 2.4 GHz.)
"""

import sys

sys.path.insert(0, "/opt/trn_rl_repo")

import numpy as np

B, L, D, N, H = 2, 2048, 1024, 16, 64
T = B * L            # 4096 tokens, batch-major
NC = 8               # cores
HPC = N // NC        # 2 heads per core
W = HPC * H          # 128 projection features per core
THETA = 10000.0
VBLK = 130           # v_sb block stride: [h0 64 | ones 1 | h1 64 | ones 1]

_CACHED = {}


def _build():
    import concourse.bass as bass
    import concourse.mybir as mybir
    import concourse.tile as tile

    F32 = mybir.dt.float32
    BF16 = mybir.dt.bfloat16
    AF = mybir.ActivationFunctionType

    # ---- fix: stock _drain_and_barrier overflows the 2-slot sync encoding
    import re as _re
    from concourse.vector_clock import ScopedClock, VectorClock

    def _split_drain_and_barrier(self, tick_clock, wait_clock):
        gc = tick_clock.global_clock
        ticks = [int(v) for v in _re.findall(r"-?\d+", str(gc))]
        for proc, t in enumerate(ticks):
            if t <= 0:
                continue
            sub = VectorClock()
            sub.require_at_least(proc, t)
            d = self.nc.sync.drain()
            wait_clock.add_sem_waits(d.ins, ScopedClock({None: sub}))
        self.nc.all_engine_barrier()
        assert self.sems is not None
        popped = self.nc._tile_sem_poison_stack.pop()
        assert popped is self._sem_poison
        self.nc.clear_and_free_semaphores(list(self.sems.allocated().values()))
        self.nc.all_engine_barrier()

    tile.TileContext._drain_and_barrier = _split_drain_and_barrier

    nc = bass.Bass()

    xT_ext = nc.declare_dram_parameter("xT", [T // 512, 128, (D // 128) * 512], BF16, isOutput=False)
    wq_ext = nc.declare_dram_parameter("wq", [128, D], BF16, isOutput=False)
    wk_ext = nc.declare_dram_parameter("wk", [128, D], BF16, isOutput=False)
    wv_ext = nc.declare_dram_parameter("wv", [128, D], BF16, isOutput=False)
    bq_ext = nc.declare_dram_parameter("bq", [W, 1], F32, isOutput=False)
    bk_ext = nc.declare_dram_parameter("bk", [W, 1], F32, isOutput=False)
    bv_ext = nc.declare_dram_parameter("bv", [W, 1], F32, isOutput=False)
    cos_ext = nc.declare_dram_parameter("cosT", [W, L], BF16, isOutput=False)
    sin_ext = nc.declare_dram_parameter("sinT", [W, L], BF16, isOutput=False)
    rmat_ext = nc.declare_dram_parameter("rmat", [128, 128], BF16, isOutput=False)
    ident_ext = nc.declare_dram_parameter("ident", [128, 128], BF16, isOutput=False)
    mask_ext = nc.declare_dram_parameter("masks", [128, 256], BF16, isOutput=False)
    ones_ext = nc.declare_dram_parameter("onesc", [128, 128], BF16, isOutput=False)
    wo_ext = nc.declare_dram_parameter("woT", [128, NC * D], BF16, isOutput=False)
    bo_ext = nc.declare_dram_parameter("bo", [1, D], BF16, isOutput=False)
    out_ext = nc.declare_dram_parameter("out", [T // NC, D], F32, isOutput=True)

    TT = T // 512      # 8 token tiles of 512
    KD = D // 128      # 8 contraction chunks
    HB = T // (2 * NC)  # 256: tokens per (core, batch-half)

    with tile.TileContext(nc) as tc, nc.allow_low_precision(reason="bf16 attn"):
        with tc.tile_pool(name="dram", bufs=1, space="DRAM") as dram:
            # dummy collective: absorbs one-time collective warmup while
            # compute runs; same shape as the real halves
            dum_in = dram.tile([NC, 128, 128], BF16)
            dum_out = dram.tile([NC, 128, 128], BF16)
            nc.gpsimd.collective_compute(
                "AllToAll", mybir.AluOpType.bypass,
                replica_groups=[list(range(NC))],
                ins=[dum_in[:].opt()], outs=[dum_out[:].opt()],
            )
            # 4 subs of 1024 tokens (128/core). A split tail (qt2/qt3 as two
            # 512-token collectives) was tried and measured neutral: the
            # early trigger is nullified because consecutive collectives
            # serialize on the CC cores, and the split doubles the tail
            # outproj matmuls. One merged tail collective is less total work.
            cins = [dram.tile([NC, 128, 128], BF16, name=f"cin{i}")
                    for i in range(2 * B)]
            couts = [dram.tile([NC, 128, 128], BF16, name=f"cout{i}")
                     for i in range(2 * B)]

            cpool = tc.alloc_tile_pool(name="const", bufs=1)
            work = tc.alloc_tile_pool(name="work", bufs=1)
            xtpool = tc.alloc_tile_pool(name="xt", bufs=4)
            t1pool = tc.alloc_tile_pool(name="p1t", bufs=3)
            atpool = tc.alloc_tile_pool(name="att", bufs=8)
            nrmpool = tc.alloc_tile_pool(name="nrm", bufs=6)
            obpool = tc.alloc_tile_pool(name="ob", bufs=4)
            psT = tc.alloc_tile_pool(name="psT", bufs=2, space="PSUM")
            pso = tc.alloc_tile_pool(name="pso", bufs=4, space="PSUM")

            # ---- long-lived working tensors (declared first: the warmup
            # and early DMAs below reference them)
            q_ro = work.tile([128, T], BF16)
            k_ro = work.tile([128, T], BF16)
            v_sb = work.tile([128, 32 * VBLK], BF16)
            o_sb = work.tile([128, T], BF16)
            og_all = work.tile([128, B * NC * HB], BF16)
            warm_sb = work.tile([128, 512], BF16)

            # ---- PE warmup: ~12 dummy matmuls on a memset tile lift the
            # HAM clock gate to 8/8 while the first input DMAs land.
            nc.vector.memset(warm_sb[:], 0.0)
            wps = pso.tile([128, 512], F32, tag="ops", name="warm")
            for _ in range(12):
                nc.tensor.matmul(wps[:], warm_sb[:, 0:128], warm_sb[:],
                                 start=True, stop=True)

            # ---- constants, spread across the engine descriptor rings
            # by first-use time so no single ring serializes the startup.
            wq_sb = cpool.tile([128, KD * 128], BF16)
            wk_sb = cpool.tile([128, KD * 128], BF16)
            wv_sb = cpool.tile([128, KD * 128], BF16)
            bq_sb = cpool.tile([W, 1], F32)
            bk_sb = cpool.tile([W, 1], F32)
            bv_sb = cpool.tile([W, 1], F32)
            rmat_sb = cpool.tile([128, 128], BF16)
            ident_sb = cpool.tile([128, 128], BF16)
            cos_sb = cpool.tile([W, L], BF16)
            sin_sb = cpool.tile([W, L], BF16)
            ones_sb = cpool.tile([128, 128], BF16)
            mask_sb = cpool.tile([128, 256], BF16)
            wo_sb = cpool.tile([128, NC * D], BF16)
            bo_sb = cpool.tile([1, D], BF16)

            # scalar ring: Q path + rope tables (halved so tt0/tt1 ropes
            # start early), then the late out-proj weights. wo must NOT
            # ride the gpsimd ring: the cin/og collective-feed DMAs would
            # queue behind its 2MB transfer and skew every AllToAll.
            nc.scalar.dma_start(rmat_sb[:], rmat_ext[:])
            nc.scalar.dma_start(wq_sb[:], wq_ext[:])
            nc.scalar.dma_start(bq_sb[:], bq_ext[:])
            nc.scalar.dma_start(cos_sb[:, 0:1024], cos_ext[:, 0:1024])
            nc.scalar.dma_start(sin_sb[:, 0:1024], sin_ext[:, 0:1024])
            nc.scalar.dma_start(cos_sb[:, 1024:L], cos_ext[:, 1024:L])
            nc.scalar.dma_start(sin_sb[:, 1024:L], sin_ext[:, 1024:L])
            # wo/bo are emitted later (after attn b0 q01) so the 2MB wo
            # transfer doesn't sit ahead of the tt2 x tile on this ring
            # gpsimd ring (after the dummy-collective trigger): K/V paths
            # + small attention constants — all done by ~25us so the ring
            # is clean when the cin/og collective DMAs start. The v_sb
            # ones columns are DVE memsets (a strided DMA here costs ~2us
            # of descriptor writes and clogs the ring head).
            v_view = v_sb[:].rearrange("p (b s) -> p b s", s=VBLK)
            nc.vector.memset(v_view[:, :, 64], 1.0)
            nc.vector.memset(v_view[:, :, 129], 1.0)
            ident1_sb = cpool.tile([1, 1], F32)
            nc.vector.memset(ident1_sb[:], 1.0)
            nc.gpsimd.dma_start(ones_sb[:], ones_ext[:])
            nc.gpsimd.dma_start(ident_sb[:], ident_ext[:])
            nc.gpsimd.dma_start(wk_sb[:], wk_ext[:])
            nc.gpsimd.dma_start(bk_sb[:], bk_ext[:])
            nc.gpsimd.dma_start(wv_sb[:], wv_ext[:])
            nc.gpsimd.dma_start(bv_sb[:], bv_ext[:])
            nc.gpsimd.dma_start(mask_sb[:], mask_ext[:])

            def emit_qkv(tt):
                xt = xtpool.tile([128, KD * 512], BF16, name="xt", tag="xt")
                if tt == 0:
                    q4 = KD * 128
                    for qq in range(4):
                        nc.sync.dma_start(xt[:, qq * q4:(qq + 1) * q4],
                                          xT_ext[tt][:, qq * q4:(qq + 1) * q4])
                elif tt in (2, 4, 5):
                    # b1's first tiles ride the scalar ring (free after the
                    # rope tables) so the sync ring isn't the only x feed
                    nc.scalar.dma_start(xt[:], xT_ext[tt])
                else:
                    nc.sync.dma_start(xt[:], xT_ext[tt])
                xts = [xt[:, k * 512:(k + 1) * 512] for k in range(KD)]
                lcol = (tt % (TT // B)) * 512

                for wsb, bsb, dst in ((wq_sb, bq_sb, q_ro),
                                      (wk_sb, bk_sb, k_ro)):
                    ps = pso.tile([128, 512], F32, tag="ops", name="pj")
                    for k in range(KD):
                        nc.tensor.matmul(ps[:], wsb[:, k * 128:(k + 1) * 128],
                                         xts[k], start=(k == 0),
                                         stop=(k == KD - 1))
                    bs = t1pool.tile([128, 512], BF16, tag="bs", name="bs")
                    # bias-add + f32->bf16 on ACT (frees DVE for the rope muls)
                    nc.scalar.activation(bs[:], ps[:], AF.Identity, bias=bsb[:])
                    rot = pso.tile([128, 512], F32, tag="ops", name="rot")
                    nc.tensor.matmul(rot[:], rmat_sb[:], bs[:],
                                     start=True, stop=True)
                    t1 = t1pool.tile([128, 512], BF16, tag="t1", name="t1")
                    nc.vector.tensor_mul(t1[:], bs[:], cos_sb[:, lcol:lcol + 512])
                    t2 = t1pool.tile([128, 512], BF16, tag="t2", name="t2")
                    nc.vector.tensor_mul(t2[:], rot[:], sin_sb[:, lcol:lcol + 512])
                    nc.vector.tensor_add(dst[:, tt * 512:(tt + 1) * 512],
                                         t1[:], t2[:])

                ps = pso.tile([128, 512], F32, tag="ops", name="pv")
                for k in range(KD):
                    nc.tensor.matmul(ps[:], wv_sb[:, k * 128:(k + 1) * 128],
                                     xts[k], start=(k == 0), stop=(k == KD - 1))
                vbs = t1pool.tile([128, 512], BF16, tag="bs", name="vbs")
                nc.scalar.activation(vbs[:], ps[:], AF.Identity, bias=bv_sb[:])
                for s in range(4):
                    vt = psT.tile([128, 128], BF16, tag="sT", name="vt")
                    nc.tensor.transpose(vt[:], vbs[:, s * 128:(s + 1) * 128],
                                        ident_sb[:])
                    blk = (tt * 4 + s) * VBLK
                    nc.vector.tensor_copy(v_sb[:, blk:blk + 64], vt[:, 0:64])
                    nc.vector.tensor_copy(v_sb[:, blk + 65:blk + 129],
                                          vt[:, 64:128])

            def emit_attention(b, qts=None):
                tof = b * L
                vb = b * (L // 128)
                for qt in (range(L // 512) if qts is None else qts):
                    nkc = 4 * qt + 4
                    opss = [pso.tile([65, 512], F32, tag="ops", name="ops")
                            for _ in range(HPC)]

                    def nwin(kc, qt=qt):
                        r = kc - 4 * qt
                        return 512 - 128 * r if r >= 0 else 512

                    def emit_sT(kc, qt=qt):
                        # diagonal chunks are column-trimmed to the causal
                        # window: queries j >= 128r within the qt block
                        n = nwin(kc)
                        qoff = tof + qt * 512 + (512 - n)
                        sT2 = psT.tile([128, 1024], F32, tag="sT", name="sT")
                        for hl in range(HPC):
                            nc.tensor.matmul(
                                sT2[:, hl * 512:hl * 512 + n],
                                k_ro[64 * hl:64 * hl + 64,
                                     tof + kc * 128:tof + kc * 128 + 128],
                                q_ro[64 * hl:64 * hl + 64, qoff:qoff + n],
                                start=True, stop=True)
                        at2 = atpool.tile([128, 1024], BF16, tag="at", name="at")
                        sv = sT2[:].rearrange("p (h n) -> p h n", h=2)
                        av = at2[:].rearrange("p (h n) -> p h n", h=2)
                        nc.scalar.activation(av[:, :, 0:n], sv[:, :, 0:n],
                                             AF.Exp)
                        if kc - 4 * qt >= 0:
                            # intra-chunk causal wedge: first 128 columns of
                            # the trimmed window, same pattern for every r
                            nc.vector.tensor_mul(
                                av[:, :, 0:128], av[:, :, 0:128],
                                mask_sb[:].rearrange("p (h n) -> p h n", h=2))
                        ats[kc] = at2

                    ats = {}

                    def emit_av(kc, first, last, opss=opss, ats=ats, vb=vb):
                        n = nwin(kc)
                        for hl in range(HPC):
                            nc.tensor.matmul(
                                opss[hl][:, 512 - n:512],
                                v_sb[:, (vb + kc) * VBLK + 65 * hl:
                                     (vb + kc) * VBLK + 65 * hl + 65],
                                ats[kc][:, hl * 512:hl * 512 + n],
                                start=(kc == first), stop=(kc == last))

                    # diagonal (masked) chunks first so no DVE mask work is
                    # queued ahead of the reciprocal at the qt boundary
                    kcs = list(range(4 * qt, nkc)) + list(range(0, 4 * qt))
                    first, last = kcs[0], kcs[-1]
                    emit_sT(first)
                    for i in range(1, nkc):
                        emit_sT(kcs[i])
                        emit_av(kcs[i - 1], first, last)
                    emit_av(last, first, last)

                    sub, c0 = b * 2 + qt // 2, 4 * (qt % 2)
                    # Boundaries that do NOT complete a sub-collective have a
                    # full qt of slack before their cin matters; push their
                    # PE-side boundary ops later (priority -= -40 makes them
                    # appear ~40 issue slots later) so the next qt's matmuls
                    # schedule ahead instead of the PE idling in-order on the
                    # ACT->PE->DVE->PE denominator chain.
                    critical = (qt % 2 == 1)
                    lazy = tc.high_priority(offset=(0 if critical else -40))
                    # Denominator reciprocals in transposed [128, 8] layout:
                    # the [1,512]-row DVE reciprocal is ~6 cycles/elem on one
                    # partition (3.3us each, 2 per qt, serial on DVE). Instead:
                    # copy both hl den rows to SBUF (ACT), PE-transpose into
                    # [128, 8], one tiny 128-way reciprocal, transpose back.
                    with lazy:
                        dens = []
                        dT = pso.tile([128, 2 * 4], F32, tag="ops", name="dT")
                        for hl in range(HPC):
                            den = nrmpool.tile([1, 512], F32, tag="den",
                                               name="den")
                            nc.scalar.activation(den[:], opss[hl][64:65, :],
                                                 AF.Identity)
                            dens.append(den)
                            for c in range(4):
                                nc.tensor.transpose(
                                    dT[:, 4 * hl + c:4 * hl + c + 1],
                                    den[:, c * 128:(c + 1) * 128],
                                    ident1_sb[:])
                        rcT = nrmpool.tile([128, 8], BF16, tag="rcT",
                                           name="rcT")
                        nc.vector.reciprocal(rcT[:], dT[:])
                        # back-transpose column-by-column so every output
                        # lands at base partition 0 (matmul operand
                        # constraint); rc1 holds [1/den] as one [1, 1024]
                        # row, hl-major chunk order
                        rcP = pso.tile([1, 1024], BF16, tag="ops", name="rcP")
                        for j in range(8):
                            nc.tensor.transpose(
                                rcP[0:1, j * 128:(j + 1) * 128],
                                rcT[:, j:j + 1], ident_sb[:])
                        rc = nrmpool.tile([1, 1024], BF16, tag="rc", name="rc")
                        nc.scalar.activation(rc[:], rcP[:], AF.Identity)
                    for hl in range(HPC):
                        hof = 64 * hl
                        ops = opss[hl]
                        with tc.high_priority(offset=(-25 if critical
                                                      else -40)):
                            bc = pso.tile([64, 512], F32, tag="ops", name="bc")
                            nc.tensor.matmul(
                                bc[:], ones_sb[0:1, 0:64],
                                rc[0:1, hl * 512:hl * 512 + 512],
                                start=True, stop=True)
                            bcs = nrmpool.tile([64, 512], F32, tag="bcs",
                                               name="bcs")
                            nc.scalar.activation(bcs[:], bc[:], AF.Identity)
                            nc.vector.tensor_mul(
                                o_sb[hof:hof + 64,
                                     tof + qt * 512:tof + qt * 512 + 512],
                                ops[0:64, :], bcs[:])
                    osl = o_sb[:, tof + qt * 512:tof + qt * 512 + 512]
                    nc.gpsimd.dma_start(
                        cins[sub][c0:c0 + 4].rearrange("c p t -> p c t"),
                        osl.rearrange("p (c t) -> p c t", c=4))

            def og_geom(sub):
                # (og/out row offset in tokens, tokens per src-core chunk)
                return sub * NC * 128, 128

            def emit_a2a(sub):
                nc.gpsimd.collective_compute(
                    "AllToAll", mybir.AluOpType.bypass,
                    replica_groups=[list(range(NC))],
                    ins=[cins[sub][:].opt()], outs=[couts[sub][:].opt()],
                )
                # og layout: [sub][srccore][tokens]; per-chunk DMAs so each
                # outproj k-loop matmul waits only its own chunk. Even chunks
                # ride gpsimd, odd ride sync: two rings halve the serial
                # drain (was 8 x ~650ns on one ring gating outproj).
                t0, w = og_geom(sub)
                for k in range(NC):
                    if sub < 3:
                        eng = nc.gpsimd if k % 2 == 0 else nc.sync
                    else:
                        # tail sub: keep the gpsimd ring clean (nothing else
                        # rides sync/scalar at that point)
                        eng = nc.sync if k % 2 == 0 else nc.scalar
                    eng.dma_start(
                        og_all[:, t0 + k * w:t0 + (k + 1) * w],
                        couts[sub][k])

            def emit_outproj(sub):
                # one token block per sub-collective
                t0, w = og_geom(sub)
                ob = obpool.tile([128, D], F32, tag="ob", name="ob")
                for half in range(2):
                    ps = psT.tile([128, 512], F32, tag="sT", name="op")
                    for k in range(NC):
                        nc.tensor.matmul(
                            ps[0:w, :],
                            og_all[:, t0 + k * w:t0 + (k + 1) * w],
                            wo_sb[:, k * D + half * 512:
                                  k * D + half * 512 + 512],
                            start=(k == 0), stop=False)
                    nc.tensor.matmul(
                        ps[0:w, :], ones_sb[0:1, 0:w],
                        bo_sb[:, half * 512:half * 512 + 512],
                        start=False, stop=True)
                    nc.scalar.activation(ob[0:w, half * 512:half * 512 + 512],
                                         ps[0:w, :], AF.Identity)
                    oeng = nc.sync if half == 0 else nc.scalar
                    oeng.dma_start(
                        out_ext[t0 // NC:t0 // NC + w,
                                half * 512:half * 512 + 512],
                        ob[0:w, half * 512:half * 512 + 512])

            # ---- interleaved schedule: sub-collectives fire as halves finish
            for tt in range(TT // B):
                with nc.named_scope(f"qkv{tt}"):
                    emit_qkv(tt)
            with nc.named_scope("attn_b0q01"):
                emit_attention(0, qts=(0, 1))
            nc.scalar.dma_start(wo_sb[:], wo_ext[:])
            nc.scalar.dma_start(bo_sb[:], bo_ext[:])
            with nc.named_scope("a2a0"):
                emit_a2a(0)
            with nc.named_scope("attn_b0q23"):
                emit_attention(0, qts=(2, 3))   # overlaps a2a(0) + b=1 qkv below
            with nc.named_scope("a2a1"):
                emit_a2a(1)
            for tt in range(TT // B, TT):
                with nc.named_scope(f"qkv{tt}"):
                    emit_qkv(tt)
            with nc.named_scope("outproj0"):
                emit_outproj(0)
            with nc.named_scope("attn_b1q01"):
                emit_attention(1, qts=(0, 1))
            with nc.named_scope("a2a2"):
                emit_a2a(2)
            with nc.named_scope("outproj1"):
                emit_outproj(1)
            with nc.named_scope("attn_b1q23"):
                emit_attention(1, qts=(2, 3))
            # outproj2 (inputs long ready) fills the PE gap while the qt3
            # boundary chain runs; emitting it before a2a3 adds no gpsimd
            # work, so the tail collective still triggers immediately
            with nc.named_scope("outproj2"):
                emit_outproj(2)
            with nc.named_scope("a2a3"):
                emit_a2a(3)                     # the tail collective
            with nc.named_scope("outproj3"):
                emit_outproj(3)

            for p in (pso, psT, obpool, nrmpool, atpool, t1pool,
                      xtpool, work, cpool):
                p.release()

    # legalize: split excess sem waits onto preceding same-engine NoOps
    import bass_rust
    from concourse import mybir as _mb
    uid = [0]
    for bb in nc.m.functions[0].blocks:
        il = bb.instructions
        todo = [i for i, inst in enumerate(il)
                if inst.sync_info is not None
                and len(inst.sync_info.on_wait) > 1]
        for idx in reversed(todo):
            inst = il[idx]
            si = inst.sync_info
            waits = list(si.on_wait)
            keep = waits[-1:]
            excess = waits[:-1]
            nops = []
            for i in range(0, len(excess)):
                uid[0] += 1
                nops.append(_mb.InstNoOp(
                    name=f"WSPLIT-{uid[0]}", engine=inst.engine, ins=[], outs=[],
                    bass_nofuse=True,
                    sync_info=bass_rust.SyncInfo(on_wait=excess[i:i + 1],
                                                 on_update=[])))
            inst.sync_info = bass_rust.SyncInfo(on_wait=keep,
                                                on_update=list(si.on_update))
            for j, nop in enumerate(nops):
                il.insert(idx + j, nop)
    return nc


def _wtile(w):
    # [W, D] -> [128, KD*128] with block k = w[:, k*128:(k+1)*128].T
    import ml_dtypes
    BF = ml_dtypes.bfloat16
    kd = w.shape[1] // 128
    return np.ascontiguousarray(
        w.reshape(128, kd, 128).transpose(2, 1, 0).reshape(128, kd * 128)
        .astype(BF))


def _host_prep(x, Wq, bq, Wk, bk, Wv, bv, Wo, bo, scale):
    import ml_dtypes
    BF = ml_dtypes.bfloat16
    s = float(np.asarray(scale).reshape(-1)[0])
    # pre-tiled layout: xT[tt, p, k*512+t] = x[tt*512+t, k*128+p]
    xr = x.reshape(T // 512, 512, D // 128, 128).astype(BF)
    xT = np.ascontiguousarray(xr.transpose(0, 3, 2, 1)
                              .reshape(T // 512, 128, (D // 128) * 512))

    # RoPE tables, feature-major, rows duplicated per interleaved pair
    freqs = THETA ** (-np.arange(0, H, 2, dtype=np.float64) / H)      # [32]
    ang = np.arange(L, dtype=np.float64)[:, None] * freqs[None, :]    # [L, 32]
    cos_t = np.repeat(np.cos(ang).T, 2, axis=0)                       # [64, L]
    sin_t = np.repeat(np.sin(ang).T, 2, axis=0)
    cosT = np.ascontiguousarray(np.tile(cos_t, (HPC, 1)).astype(BF))
    sinT = np.ascontiguousarray(np.tile(sin_t, (HPC, 1)).astype(BF))

    rmat = np.zeros((128, 128), dtype=BF)
    for i in range(64):
        rmat[2 * i + 1, 2 * i] = -1.0
        rmat[2 * i, 2 * i + 1] = 1.0

    ident = np.eye(128, dtype=BF)
    onesc = np.ones((128, 128), dtype=BF)

    # causal wedge for the first 128 columns of every trimmed diagonal
    # chunk: keep (k <= j'), tiled for the two heads
    kt = np.arange(128)[:, None]
    jc = np.arange(128)[None, :]
    wedge = np.where(kt <= jc, 1.0, 0.0).astype(BF)
    masks = np.ascontiguousarray(np.tile(wedge, (1, 2)))              # [128, 256]

    woT = np.ascontiguousarray(
        Wo.T.astype(BF).reshape(NC, 128, D).transpose(1, 0, 2)
        .reshape(128, NC * D))
    bo_row = np.ascontiguousarray(bo.astype(BF).reshape(1, D))

    Wq_s = (Wq * s).astype(np.float32)
    bq_s = (bq * s).astype(np.float32)

    in_maps = []
    for c in range(NC):
        hsl = slice(c * W, (c + 1) * W)
        in_maps.append({
            "xT": xT,
            "wq": _wtile(Wq_s[hsl, :]),
            "wk": _wtile(Wk[hsl, :]),
            "wv": _wtile(Wv[hsl, :]),
            "bq": np.ascontiguousarray(bq_s[hsl].reshape(W, 1)),
            "bk": np.ascontiguousarray(bk[hsl].astype(np.float32).reshape(W, 1)),
            "bv": np.ascontiguousarray(bv[hsl].astype(np.float32).reshape(W, 1)),
            "cosT": cosT, "sinT": sinT, "rmat": rmat, "ident": ident,
            "masks": masks, "onesc": onesc, "woT": woT, "bo": bo_row,
        })
    return in_maps


def kernel(x, Wq, bq, Wk, bk, Wv, bv, Wo, bo, scale):
    from concourse.bass_utils import run_bass_kernel_spmd

    if "nc" not in _CACHED:
        _CACHED["nc"] = _build()
    nc = _CACHED["nc"]
    in_maps = _host_prep(np.asarray(x), np.asarray(Wq), np.asarray(bq),
                         np.asarray(Wk), np.asarray(bk), np.asarray(Wv),
                         np.asarray(bv), np.asarray(Wo), np.asarray(bo),
                         np.asarray(scale))
    res = run_bass_kernel_spmd(nc, in_maps, list(range(NC)))
    return _assemble(res)


def _assemble(res):
    out = np.empty((T, D), dtype=np.float32)
    for c in range(NC):
        r = res.results[c]["out"]
        for sub in range(4):
            b, s = divmod(sub, 2)
            t0 = b * L + s * 1024 + 128 * c
            out[t0:t0 + 128] = r[sub * 128:(sub + 1) * 128]
    return out.reshape(B, L, D).astype(np.float32)



# revision 47
# speedup vs baseline: 1.0744x; 1.0744x over previous
"""Causal multi-head attention (RoPE) on 8 TRN2 NeuronCores.

Sharding: Megatron-style head parallelism. Each core owns 2 of the 16 heads:
it computes q/k/v projections for its 128 output features (2 heads x 64),
applies interleaved-pair RoPE (rotation done as a PE matmul with a constant
pair-swap matrix R, tables precomputed host-side), runs causal attention for
its (batch, head) pairs in the transposed orientation S^T = K^T Q so that no
on-chip transposes of the attention matrix are needed, and accumulates
attn^T-weighted V with an extra all-ones V column that yields the softmax
denominators for free. An AllToAll then redistributes the per-head outputs
from head-sharded to token-sharded layout, and each core computes the final
output projection for its 512-token slice. A tiny dummy AllToAll is issued at
kernel start to absorb the one-time collective warmup (~95us) while the
engines compute.

Perf structure (v3):
- Diagonal S^T/exp/AV chunks are column-trimmed to the causal window
  (queries >= first key of the chunk), cutting ACT-exp time and PE stream
  time; the residual intra-chunk wedge mask is a single shared [128,256]
  constant applied with a 3D AP.
- Softmax denominators: the [1,512]-row DVE reciprocal costs ~6 cyc/elem
  on a single partition (3.3us, 2x per qt, serial on DVE). Instead both
  hl den rows are ACT-copied to SBUF, PE-transposed into [128,8], one
  128-way-parallel reciprocal (~0.2us), transposed back column-by-column
  into a [1,1024] row (base-partition-0 constraint) feeding the bc
  broadcast matmuls.
- QKV bias-adds ride the ACT bias port (frees DVE for the rope muls).
- 4 sub-collectives of 1024 tokens. outproj2 is emitted before the tail
  collective so the scheduler interleaves it with qt3; tail og drains
  ride sync+scalar, keeping the gpsimd ring free for cin feeds +
  collective triggers. The remaining tail is bounded by ncfw a2a latency
  (~17-25us/collective, serialized on the CC cores; a split tail
  measured neutral because of that serialization).
- Per-qt AllToAll feed DMAs keep a dedicated gpsimd ring; the 2MB wo
  transfer is emitted after b0's first attention block so it doesn't sit
  ahead of the tt2 x tile on the scalar ring.
- Constant DMAs are spread across the scalar/gpsimd/sync queues so the
  first QKV matmul isn't gated on one serialized descriptor ring, and a
  burst of dummy matmuls at t=0 lifts the PE HAM clock gate to 2.4 GHz
  before real work lands. (Sustained rate is HAM power-throttled to
  ~50-81% duty for most of the run; total PE work is the binding floor.)
- PSUM: 2x[128,1024] sT pipeline (4 banks) + 4x[128,512] shared pool
  (attention accumulators, rot/bc/dT/rcP transients, QKV projection
  accumulator).

Compute dtype: bf16 operands with fp32 PSUM accumulation. (float32r would
be more precise, but f32r matmuls do not register as PE activity for the HAM
clock gate, so the PE stays throttled at 1.2 GHz; bf16 runs at 2.4 GHz.)
"""

import sys

sys.path.insert(0, "/opt/trn_rl_repo")

import numpy as np

B, L, D, N, H = 2, 2048, 1024, 16, 64
T = B * L            # 4096 tokens, batch-major
NC = 8               # cores
HPC = N // NC        # 2 heads per core
W = HPC * H          # 128 projection features per core
THETA = 10000.0
VBLK = 130           # v_sb block stride: [h0 64 | ones 1 | h1 64 | ones 1]

_CACHED = {}


def _build():
    import concourse.bass as bass
    import concourse.mybir as mybir
    import concourse.tile as tile

    F32 = mybir.dt.float32
    BF16 = mybir.dt.bfloat16
    AF = mybir.ActivationFunctionType

    # ---- fix: stock _drain_and_barrier overflows the 2-slot sync encoding
    import re as _re
    from concourse.vector_clock import ScopedClock, VectorClock

    def _split_drain_and_barrier(self, tick_clock, wait_clock):
        gc = tick_clock.global_clock
        ticks = [int(v) for v in _re.findall(r"-?\d+", str(gc))]
        for proc, t in enumerate(ticks):
            if t <= 0:
                continue
            sub = VectorClock()
            sub.require_at_least(proc, t)
            d = self.nc.sync.drain()
            wait_clock.add_sem_waits(d.ins, ScopedClock({None: sub}))
        self.nc.all_engine_barrier()
        assert self.sems is not None
        popped = self.nc._tile_sem_poison_stack.pop()
        assert popped is self._sem_poison
        self.nc.clear_and_free_semaphores(list(self.sems.allocated().values()))
        self.nc.all_engine_barrier()

    tile.TileContext._drain_and_barrier = _split_drain_and_barrier

    nc = bass.Bass()

    xT_ext = nc.declare_dram_parameter("xT", [T // 512, 128, (D // 128) * 512], BF16, isOutput=False)
    wq_ext = nc.declare_dram_parameter("wq", [128, D], BF16, isOutput=False)
    wk_ext = nc.declare_dram_parameter("wk", [128, D], BF16, isOutput=False)
    wv_ext = nc.declare_dram_parameter("wv", [128, D], BF16, isOutput=False)
    bq_ext = nc.declare_dram_parameter("bq", [W, 1], F32, isOutput=False)
    bk_ext = nc.declare_dram_parameter("bk", [W, 1], F32, isOutput=False)
    bv_ext = nc.declare_dram_parameter("bv", [W, 1], F32, isOutput=False)
    cos_ext = nc.declare_dram_parameter("cosT", [W, L], BF16, isOutput=False)
    sin_ext = nc.declare_dram_parameter("sinT", [W, L], BF16, isOutput=False)
    rmat_ext = nc.declare_dram_parameter("rmat", [128, 128], BF16, isOutput=False)
    ident_ext = nc.declare_dram_parameter("ident", [128, 128], BF16, isOutput=False)
    mask_ext = nc.declare_dram_parameter("masks", [128, 256], BF16, isOutput=False)
    ones_ext = nc.declare_dram_parameter("onesc", [128, 128], BF16, isOutput=False)
    wo_ext = nc.declare_dram_parameter("woT", [128, NC * D], BF16, isOutput=False)
    bo_ext = nc.declare_dram_parameter("bo", [1, D], BF16, isOutput=False)
    out_ext = nc.declare_dram_parameter("out", [T // NC, D], F32, isOutput=True)

    TT = T // 512      # 8 token tiles of 512
    KD = D // 128      # 8 contraction chunks
    HB = T // (2 * NC)  # 256: tokens per (core, batch-half)

    with tile.TileContext(nc) as tc, nc.allow_low_precision(reason="bf16 attn"):
        with tc.tile_pool(name="dram", bufs=1, space="DRAM") as dram:
            # dummy collective (trigger emitted below, after the gpsimd
            # const DMA dispatches): absorbs one-time collective warmup
            # while compute runs; same shape as the real halves
            dum_in = dram.tile([NC, 128, 128], BF16)
            dum_out = dram.tile([NC, 128, 128], BF16)
            # 4 subs of 1024 tokens (128/core). A split tail (qt2/qt3 as two
            # 512-token collectives) was tried and measured neutral: the
            # early trigger is nullified because consecutive collectives
            # serialize on the CC cores, and the split doubles the tail
            # outproj matmuls. One merged tail collective is less total work.
            cins = [dram.tile([NC, 128, 128], BF16, name=f"cin{i}")
                    for i in range(2 * B)]
            couts = [dram.tile([NC, 128, 128], BF16, name=f"cout{i}")
                     for i in range(2 * B)]

            cpool = tc.alloc_tile_pool(name="const", bufs=1)
            work = tc.alloc_tile_pool(name="work", bufs=1)
            xtpool = tc.alloc_tile_pool(name="xt", bufs=4)
            t1pool = tc.alloc_tile_pool(name="p1t", bufs=3)
            atpool = tc.alloc_tile_pool(name="att", bufs=8)
            nrmpool = tc.alloc_tile_pool(name="nrm", bufs=6)
            obpool = tc.alloc_tile_pool(name="ob", bufs=4)
            psT = tc.alloc_tile_pool(name="psT", bufs=2, space="PSUM")
            pso = tc.alloc_tile_pool(name="pso", bufs=4, space="PSUM")

            # ---- long-lived working tensors (declared first: the warmup
            # and early DMAs below reference them)
            q_ro = work.tile([128, T], BF16)
            k_ro = work.tile([128, T], BF16)
            v_sb = work.tile([128, 32 * VBLK], BF16)
            o_sb = work.tile([128, T], BF16)
            og_all = work.tile([128, B * NC * HB], BF16)
            warm_sb = work.tile([128, 512], BF16)

            # ---- PE warmup: ~12 dummy matmuls on a memset tile lift the
            # HAM clock gate to 8/8 while the first input DMAs land.
            nc.vector.memset(warm_sb[:], 0.0)
            wps = pso.tile([128, 512], F32, tag="ops", name="warm")
            for _ in range(12):
                nc.tensor.matmul(wps[:], warm_sb[:, 0:128], warm_sb[:],
                                 start=True, stop=True)

            # ---- constants, spread across the engine descriptor rings
            # by first-use time so no single ring serializes the startup.
            wq_sb = cpool.tile([128, KD * 128], BF16)
            wk_sb = cpool.tile([128, KD * 128], BF16)
            wv_sb = cpool.tile([128, KD * 128], BF16)
            bq_sb = cpool.tile([W, 1], F32)
            bk_sb = cpool.tile([W, 1], F32)
            bv_sb = cpool.tile([W, 1], F32)
            rmat_sb = cpool.tile([128, 128], BF16)
            ident_sb = cpool.tile([128, 128], BF16)
            cos_sb = cpool.tile([W, L], BF16)
            sin_sb = cpool.tile([W, L], BF16)
            ones_sb = cpool.tile([128, 128], BF16)
            mask_sb = cpool.tile([128, 256], BF16)
            wo_sb = cpool.tile([128, NC * D], BF16)
            bo_sb = cpool.tile([1, D], BF16)

            # scalar ring: Q path + rope tables (halved so tt0/tt1 ropes
            # start early), then the late out-proj weights. wo must NOT
            # ride the gpsimd ring: the cin/og collective-feed DMAs would
            # queue behind its 2MB transfer and skew every AllToAll.
            nc.scalar.dma_start(rmat_sb[:], rmat_ext[:])
            nc.scalar.dma_start(wq_sb[:], wq_ext[:])
            nc.scalar.dma_start(bq_sb[:], bq_ext[:])
            nc.scalar.dma_start(cos_sb[:, 0:1024], cos_ext[:, 0:1024])
            nc.scalar.dma_start(sin_sb[:, 0:1024], sin_ext[:, 0:1024])
            nc.scalar.dma_start(cos_sb[:, 1024:L], cos_ext[:, 1024:L])
            nc.scalar.dma_start(sin_sb[:, 1024:L], sin_ext[:, 1024:L])
            # wo/bo are emitted later (after attn b0 q01) so the 2MB wo
            # transfer doesn't sit ahead of the tt2 x tile on this ring
            # gpsimd ring (after the dummy-collective trigger): K/V paths
            # + small attention constants — all done by ~25us so the ring
            # is clean when the cin/og collective DMAs start. The v_sb
            # ones columns are DVE memsets (a strided DMA here costs ~2us
            # of descriptor writes and clogs the ring head).
            v_view = v_sb[:].rearrange("p (b s) -> p b s", s=VBLK)
            nc.vector.memset(v_view[:, :, 64], 1.0)
            nc.vector.memset(v_view[:, :, 129], 1.0)
            ident1_sb = cpool.tile([1, 1], F32)
            nc.vector.memset(ident1_sb[:], 1.0)
            nc.gpsimd.dma_start(ones_sb[:], ones_ext[:])
            nc.gpsimd.dma_start(ident_sb[:], ident_ext[:])
            nc.gpsimd.dma_start(wk_sb[:], wk_ext[:])
            nc.gpsimd.dma_start(bk_sb[:], bk_ext[:])
            nc.gpsimd.dma_start(wv_sb[:], wv_ext[:])
            nc.gpsimd.dma_start(bv_sb[:], bv_ext[:])
            nc.gpsimd.dma_start(mask_sb[:], mask_ext[:])
            # warmup collective AFTER the K/V-path const dispatches: its
            # trigger stalls the gpsimd sequencer during CC warmup, which
            # used to delay wk/wv landing to ~23us and stall tt0's
            # K-projection by ~3us
            nc.gpsimd.collective_compute(
                "AllToAll", mybir.AluOpType.bypass,
                replica_groups=[list(range(NC))],
                ins=[dum_in[:].opt()], outs=[dum_out[:].opt()],
            )

            def emit_qkv(tt):
                xt = xtpool.tile([128, KD * 512], BF16, name="xt", tag="xt")
                if tt == 0:
                    q4 = KD * 128
                    for qq in range(4):
                        nc.sync.dma_start(xt[:, qq * q4:(qq + 1) * q4],
                                          xT_ext[tt][:, qq * q4:(qq + 1) * q4])
                elif tt in (2, 4, 5):
                    # b1's first tiles ride the scalar ring (free after the
                    # rope tables) so the sync ring isn't the only x feed
                    nc.scalar.dma_start(xt[:], xT_ext[tt])
                else:
                    nc.sync.dma_start(xt[:], xT_ext[tt])
                xts = [xt[:, k * 512:(k + 1) * 512] for k in range(KD)]
                lcol = (tt % (TT // B)) * 512

                for wsb, bsb, dst in ((wq_sb, bq_sb, q_ro),
                                      (wk_sb, bk_sb, k_ro)):
                    ps = pso.tile([128, 512], F32, tag="ops", name="pj")
                    for k in range(KD):
                        nc.tensor.matmul(ps[:], wsb[:, k * 128:(k + 1) * 128],
                                         xts[k], start=(k == 0),
                                         stop=(k == KD - 1))
                    bs = t1pool.tile([128, 512], BF16, tag="bs", name="bs")
                    # bias-add + f32->bf16 on ACT (frees DVE for the rope muls)
                    nc.scalar.activation(bs[:], ps[:], AF.Identity, bias=bsb[:])
                    rot = pso.tile([128, 512], F32, tag="ops", name="rot")
                    nc.tensor.matmul(rot[:], rmat_sb[:], bs[:],
                                     start=True, stop=True)
                    t1 = t1pool.tile([128, 512], BF16, tag="t1", name="t1")
                    nc.vector.tensor_mul(t1[:], bs[:], cos_sb[:, lcol:lcol + 512])
                    t2 = t1pool.tile([128, 512], BF16, tag="t2", name="t2")
                    nc.vector.tensor_mul(t2[:], rot[:], sin_sb[:, lcol:lcol + 512])
                    nc.vector.tensor_add(dst[:, tt * 512:(tt + 1) * 512],
                                         t1[:], t2[:])

                ps = pso.tile([128, 512], F32, tag="ops", name="pv")
                for k in range(KD):
                    nc.tensor.matmul(ps[:], wv_sb[:, k * 128:(k + 1) * 128],
                                     xts[k], start=(k == 0), stop=(k == KD - 1))
                vbs = t1pool.tile([128, 512], BF16, tag="bs", name="vbs")
                nc.scalar.activation(vbs[:], ps[:], AF.Identity, bias=bv_sb[:])
                for s in range(4):
                    vt = psT.tile([128, 128], BF16, tag="sT", name="vt")
                    nc.tensor.transpose(vt[:], vbs[:, s * 128:(s + 1) * 128],
                                        ident_sb[:])
                    blk = (tt * 4 + s) * VBLK
                    nc.vector.tensor_copy(v_sb[:, blk:blk + 64], vt[:, 0:64])
                    nc.vector.tensor_copy(v_sb[:, blk + 65:blk + 129],
                                          vt[:, 64:128])

            def emit_attention(b, qts=None):
                tof = b * L
                vb = b * (L // 128)
                for qt in (range(L // 512) if qts is None else qts):
                    nkc = 4 * qt + 4
                    opss = [pso.tile([65, 512], F32, tag="ops", name="ops")
                            for _ in range(HPC)]

                    def nwin(kc, qt=qt):
                        r = kc - 4 * qt
                        return 512 - 128 * r if r >= 0 else 512

                    def emit_sT(kc, qt=qt):
                        # diagonal chunks are column-trimmed to the causal
                        # window: queries j >= 128r within the qt block
                        n = nwin(kc)
                        qoff = tof + qt * 512 + (512 - n)
                        sT2 = psT.tile([128, 1024], F32, tag="sT", name="sT")
                        for hl in range(HPC):
                            nc.tensor.matmul(
                                sT2[:, hl * 512:hl * 512 + n],
                                k_ro[64 * hl:64 * hl + 64,
                                     tof + kc * 128:tof + kc * 128 + 128],
                                q_ro[64 * hl:64 * hl + 64, qoff:qoff + n],
                                start=True, stop=True)
                        at2 = atpool.tile([128, 1024], BF16, tag="at", name="at")
                        sv = sT2[:].rearrange("p (h n) -> p h n", h=2)
                        av = at2[:].rearrange("p (h n) -> p h n", h=2)
                        nc.scalar.activation(av[:, :, 0:n], sv[:, :, 0:n],
                                             AF.Exp)
                        if kc - 4 * qt >= 0:
                            # intra-chunk causal wedge: first 128 columns of
                            # the trimmed window, same pattern for every r
                            nc.vector.tensor_mul(
                                av[:, :, 0:128], av[:, :, 0:128],
                                mask_sb[:].rearrange("p (h n) -> p h n", h=2))
                        ats[kc] = at2

                    ats = {}

                    def emit_av(kc, first, last, opss=opss, ats=ats, vb=vb):
                        n = nwin(kc)
                        for hl in range(HPC):
                            nc.tensor.matmul(
                                opss[hl][:, 512 - n:512],
                                v_sb[:, (vb + kc) * VBLK + 65 * hl:
                                     (vb + kc) * VBLK + 65 * hl + 65],
                                ats[kc][:, hl * 512:hl * 512 + n],
                                start=(kc == first), stop=(kc == last))

                    # diagonal (masked) chunks first so no DVE mask work is
                    # queued ahead of the reciprocal at the qt boundary
                    kcs = list(range(4 * qt, nkc)) + list(range(0, 4 * qt))
                    first, last = kcs[0], kcs[-1]
                    emit_sT(first)
                    for i in range(1, nkc):
                        emit_sT(kcs[i])
                        emit_av(kcs[i - 1], first, last)
                    emit_av(last, first, last)

                    sub, c0 = b * 2 + qt // 2, 4 * (qt % 2)
                    # Boundaries that do NOT complete a sub-collective have a
                    # full qt of slack before their cin matters; push their
                    # PE-side boundary ops later (priority -= -40 makes them
                    # appear ~40 issue slots later) so the next qt's matmuls
                    # schedule ahead instead of the PE idling in-order on the
                    # ACT->PE->DVE->PE denominator chain.
                    critical = (qt % 2 == 1)
                    lazy = tc.high_priority(offset=(0 if critical else -40))
                    # Denominator reciprocals in transposed [128, 8] layout:
                    # the [1,512]-row DVE reciprocal is ~6 cycles/elem on one
                    # partition (3.3us each, 2 per qt, serial on DVE). Instead:
                    # copy both hl den rows to SBUF (ACT), PE-transpose into
                    # [128, 8], one tiny 128-way reciprocal, transpose back.
                    with lazy:
                        dens = []
                        dT = pso.tile([128, 2 * 4], F32, tag="ops", name="dT")
                        for hl in range(HPC):
                            den = nrmpool.tile([1, 512], F32, tag="den",
                                               name="den")
                            nc.scalar.activation(den[:], opss[hl][64:65, :],
                                                 AF.Identity)
                            dens.append(den)
                            for c in range(4):
                                nc.tensor.transpose(
                                    dT[:, 4 * hl + c:4 * hl + c + 1],
                                    den[:, c * 128:(c + 1) * 128],
                                    ident1_sb[:])
                        rcT = nrmpool.tile([128, 8], BF16, tag="rcT",
                                           name="rcT")
                        nc.vector.reciprocal(rcT[:], dT[:])
                        # back-transpose column-by-column so every output
                        # lands at base partition 0 (matmul operand
                        # constraint); rc1 holds [1/den] as one [1, 1024]
                        # row, hl-major chunk order
                        rcP = pso.tile([1, 1024], BF16, tag="ops", name="rcP")
                        for j in range(8):
                            nc.tensor.transpose(
                                rcP[0:1, j * 128:(j + 1) * 128],
                                rcT[:, j:j + 1], ident_sb[:])
                        rc = nrmpool.tile([1, 1024], BF16, tag="rc", name="rc")
                        nc.scalar.activation(rc[:], rcP[:], AF.Identity)
                    for hl in range(HPC):
                        hof = 64 * hl
                        ops = opss[hl]
                        with tc.high_priority(offset=(-25 if critical
                                                      else -40)):
                            bc = pso.tile([64, 512], F32, tag="ops", name="bc")
                            nc.tensor.matmul(
                                bc[:], ones_sb[0:1, 0:64],
                                rc[0:1, hl * 512:hl * 512 + 512],
                                start=True, stop=True)
                            bcs = nrmpool.tile([64, 512], F32, tag="bcs",
                                               name="bcs")
                            nc.scalar.activation(bcs[:], bc[:], AF.Identity)
                            nc.vector.tensor_mul(
                                o_sb[hof:hof + 64,
                                     tof + qt * 512:tof + qt * 512 + 512],
                                ops[0:64, :], bcs[:])
                    osl = o_sb[:, tof + qt * 512:tof + qt * 512 + 512]
                    nc.gpsimd.dma_start(
                        cins[sub][c0:c0 + 4].rearrange("c p t -> p c t"),
                        osl.rearrange("p (c t) -> p c t", c=4))

            def og_geom(sub):
                # (og/out row offset in tokens, tokens per src-core chunk)
                return sub * NC * 128, 128

            def emit_a2a(sub):
                nc.gpsimd.collective_compute(
                    "AllToAll", mybir.AluOpType.bypass,
                    replica_groups=[list(range(NC))],
                    ins=[cins[sub][:].opt()], outs=[couts[sub][:].opt()],
                )
                # og layout: [sub][srccore][tokens]; per-chunk DMAs so each
                # outproj k-loop matmul waits only its own chunk. Even chunks
                # ride gpsimd, odd ride sync: two rings halve the serial
                # drain (was 8 x ~650ns on one ring gating outproj).
                t0, w = og_geom(sub)
                for k in range(NC):
                    if sub < 3:
                        eng = nc.gpsimd if k % 2 == 0 else nc.sync
                    else:
                        # tail sub: keep the gpsimd ring clean (nothing else
                        # rides sync/scalar at that point)
                        eng = nc.sync if k % 2 == 0 else nc.scalar
                    eng.dma_start(
                        og_all[:, t0 + k * w:t0 + (k + 1) * w],
                        couts[sub][k])

            def emit_outproj(sub):
                # one token block per sub-collective
                t0, w = og_geom(sub)
                ob = obpool.tile([128, D], F32, tag="ob", name="ob")
                for half in range(2):
                    ps = psT.tile([128, 512], F32, tag="sT", name="op")
                    for k in range(NC):
                        nc.tensor.matmul(
                            ps[0:w, :],
                            og_all[:, t0 + k * w:t0 + (k + 1) * w],
                            wo_sb[:, k * D + half * 512:
                                  k * D + half * 512 + 512],
                            start=(k == 0), stop=False)
                    nc.tensor.matmul(
                        ps[0:w, :], ones_sb[0:1, 0:w],
                        bo_sb[:, half * 512:half * 512 + 512],
                        start=False, stop=True)
                    nc.scalar.activation(ob[0:w, half * 512:half * 512 + 512],
                                         ps[0:w, :], AF.Identity)
                    oeng = nc.sync if half == 0 else nc.scalar
                    oeng.dma_start(
                        out_ext[t0 // NC:t0 // NC + w,
                                half * 512:half * 512 + 512],
                        ob[0:w, half * 512:half * 512 + 512])

            # ---- interleaved schedule: sub-collectives fire as halves finish
            for tt in range(TT // B):
                with nc.named_scope(f"qkv{tt}"):
                    emit_qkv(tt)
            with nc.named_scope("attn_b0q01"):
                emit_attention(0, qts=(0, 1))
            nc.scalar.dma_start(wo_sb[:], wo_ext[:])
            nc.scalar.dma_start(bo_sb[:], bo_ext[:])
            with nc.named_scope("a2a0"):
                emit_a2a(0)
            with nc.named_scope("attn_b0q23"):
                emit_attention(0, qts=(2, 3))   # overlaps a2a(0) + b=1 qkv below
            with nc.named_scope("a2a1"):
                emit_a2a(1)
            for tt in range(TT // B, TT):
                with nc.named_scope(f"qkv{tt}"):
                    emit_qkv(tt)
            with nc.named_scope("outproj0"):
                emit_outproj(0)
            with nc.named_scope("attn_b1q01"):
                emit_attention(1, qts=(0, 1))
            with nc.named_scope("a2a2"):
                emit_a2a(2)
            with nc.named_scope("outproj1"):
                emit_outproj(1)
            with nc.named_scope("attn_b1q23"):
                emit_attention(1, qts=(2, 3))
            # outproj2 (inputs long ready) fills the PE gap while the qt3
            # boundary chain runs; emitting it before a2a3 adds no gpsimd
            # work, so the tail collective still triggers immediately
            with nc.named_scope("outproj2"):
                emit_outproj(2)
            with nc.named_scope("a2a3"):
                emit_a2a(3)                     # the tail collective
            with nc.named_scope("outproj3"):
                emit_outproj(3)

            for p in (pso, psT, obpool, nrmpool, atpool, t1pool,
                      xtpool, work, cpool):
                p.release()

    # legalize: split excess sem waits onto preceding same-engine NoOps
    import bass_rust
    from concourse import mybir as _mb
    uid = [0]
    for bb in nc.m.functions[0].blocks:
        il = bb.instructions
        todo = [i for i, inst in enumerate(il)
                if inst.sync_info is not None
                and len(inst.sync_info.on_wait) > 1]
        for idx in reversed(todo):
            inst = il[idx]
            si = inst.sync_info
            waits = list(si.on_wait)
            keep = waits[-1:]
            excess = waits[:-1]
            nops = []
            for i in range(0, len(excess)):
                uid[0] += 1
                nops.append(_mb.InstNoOp(
                    name=f"WSPLIT-{uid[0]}", engine=inst.engine, ins=[], outs=[],
                    bass_nofuse=True,
                    sync_info=bass_rust.SyncInfo(on_wait=excess[i:i + 1],
                                                 on_update=[])))
            inst.sync_info = bass_rust.SyncInfo(on_wait=keep,
                                                on_update=list(si.on_update))
            for j, nop in enumerate(nops):
                il.insert(idx + j, nop)
    return nc


def _wtile(w):
    # [W, D] -> [128, KD*128] with block k = w[:, k*128:(k+1)*128].T
    import ml_dtypes
    BF = ml_dtypes.bfloat16
    kd = w.shape[1] // 128
    return np.ascontiguousarray(
        w.reshape(128, kd, 128).transpose(2, 1, 0).reshape(128, kd * 128)
        .astype(BF))


def _host_prep(x, Wq, bq, Wk, bk, Wv, bv, Wo, bo, scale):
    import ml_dtypes
    BF = ml_dtypes.bfloat16
    s = float(np.asarray(scale).reshape(-1)[0])
    # pre-tiled layout: xT[tt, p, k*512+t] = x[tt*512+t, k*128+p]
    xr = x.reshape(T // 512, 512, D // 128, 128).astype(BF)
    xT = np.ascontiguousarray(xr.transpose(0, 3, 2, 1)
                              .reshape(T // 512, 128, (D // 128) * 512))

    # RoPE tables, feature-major, rows duplicated per interleaved pair
    freqs = THETA ** (-np.arange(0, H, 2, dtype=np.float64) / H)      # [32]
    ang = np.arange(L, dtype=np.float64)[:, None] * freqs[None, :]    # [L, 32]
    cos_t = np.repeat(np.cos(ang).T, 2, axis=0)                       # [64, L]
    sin_t = np.repeat(np.sin(ang).T, 2, axis=0)
    cosT = np.ascontiguousarray(np.tile(cos_t, (HPC, 1)).astype(BF))
    sinT = np.ascontiguousarray(np.tile(sin_t, (HPC, 1)).astype(BF))

    rmat = np.zeros((128, 128), dtype=BF)
    for i in range(64):
        rmat[2 * i + 1, 2 * i] = -1.0
        rmat[2 * i, 2 * i + 1] = 1.0

    ident = np.eye(128, dtype=BF)
    onesc = np.ones((128, 128), dtype=BF)

    # causal wedge for the first 128 columns of every trimmed diagonal
    # chunk: keep (k <= j'), tiled for the two heads
    kt = np.arange(128)[:, None]
    jc = np.arange(128)[None, :]
    wedge = np.where(kt <= jc, 1.0, 0.0).astype(BF)
    masks = np.ascontiguousarray(np.tile(wedge, (1, 2)))              # [128, 256]

    woT = np.ascontiguousarray(
        Wo.T.astype(BF).reshape(NC, 128, D).transpose(1, 0, 2)
        .reshape(128, NC * D))
    bo_row = np.ascontiguousarray(bo.astype(BF).reshape(1, D))

    Wq_s = (Wq * s).astype(np.float32)
    bq_s = (bq * s).astype(np.float32)

    in_maps = []
    for c in range(NC):
        hsl = slice(c * W, (c + 1) * W)
        in_maps.append({
            "xT": xT,
            "wq": _wtile(Wq_s[hsl, :]),
            "wk": _wtile(Wk[hsl, :]),
            "wv": _wtile(Wv[hsl, :]),
            "bq": np.ascontiguousarray(bq_s[hsl].reshape(W, 1)),
            "bk": np.ascontiguousarray(bk[hsl].astype(np.float32).reshape(W, 1)),
            "bv": np.ascontiguousarray(bv[hsl].astype(np.float32).reshape(W, 1)),
            "cosT": cosT, "sinT": sinT, "rmat": rmat, "ident": ident,
            "masks": masks, "onesc": onesc, "woT": woT, "bo": bo_row,
        })
    return in_maps


def kernel(x, Wq, bq, Wk, bk, Wv, bv, Wo, bo, scale):
    from concourse.bass_utils import run_bass_kernel_spmd

    if "nc" not in _CACHED:
        _CACHED["nc"] = _build()
    nc = _CACHED["nc"]
    in_maps = _host_prep(np.asarray(x), np.asarray(Wq), np.asarray(bq),
                         np.asarray(Wk), np.asarray(bk), np.asarray(Wv),
                         np.asarray(bv), np.asarray(Wo), np.asarray(bo),
                         np.asarray(scale))
    res = run_bass_kernel_spmd(nc, in_maps, list(range(NC)))
    return _assemble(res)


def _assemble(res):
    out = np.empty((T, D), dtype=np.float32)
    for c in range(NC):
        r = res.results[c]["out"]
        for sub in range(4):
            b, s = divmod(sub, 2)
            t0 = b * L + s * 1024 + 128 * c
            out[t0:t0 + 128] = r[sub * 128:(sub + 1) * 128]
    return out.reshape(B, L, D).astype(np.float32)

